# revision 39
# baseline (speedup 1.0000x reference)
"""Trainium2 Bass kernel for batched dot-product attention that also returns
the attention-probability matrix.

reference:
    z = einsum("bqd,bkd->bqk", q, k) / sqrt(d)
    a = softmax(z, axis=-1)            # [B, Q, K]  (also an output!)
    v = einsum("bqk,bkd->bqd", a, v)   # [B, Q, D]
    returns (v, a)

Shapes: B=32, Q=K=2048, D=128, fp32 in/out.  8 NeuronCores, batch-sharded
4 batches/core (fully data parallel, no collectives).

Device-side layout trick: everything is computed in the "S^T" layout
(S^T[k, q] tiles with k on partitions) so that
  - the first matmul is  S^T = (K^T)^T-stationary @ Q^T-moving,
  - exp(S^T) tiles feed the A@V matmul directly as stationary operands
    (contraction over k = partitions) with V as the moving operand,
  - an extra all-ones column appended to V yields the softmax denominators
    in the same accumulation (flash-attention style),
  - A^T is written to DRAM [b, k, q]-major and transposed to [b, q, k] on the
    host as a zero-copy numpy view during the gather step.
Q/K arrive pre-transposed (and bf16-cast) from the host staging step, V
arrives bf16; all matmuls run in bf16 (f32 PSUM accumulation).
"""

import math
import os
import sys

sys.path.insert(0, "/opt/trn_rl_repo")

import numpy as np
import ml_dtypes

from contextlib import ExitStack

import concourse.bass as bass  # noqa: F401  (engine types come via nc)
import concourse.tile as tile
from concourse import bacc, mybir
from concourse.bass_utils import run_bass_kernel_spmd
from concourse.masks import make_identity

BF16 = mybir.dt.bfloat16
F32 = mybir.dt.float32

N_CORES = 8
B_FULL, SEQ_FULL, D_FULL = 32, 2048, 128
BPC_FULL = B_FULL // N_CORES  # batches per core

LAST_RESULTS = None  # BassKernelResults of the most recent kernel() call


def build_attention(
    nc,
    BPC,
    SEQ,
    D,
    QC,
    first_qc=None,
    interleave_av=False,
    last_split=True,
    ps_s_bufs=2,
    ps_o_bufs=4,
):
    """Emit the per-core attention kernel into `nc`.

    BPC: batches per core, SEQ: q==k length, D: head dim (must be 128),
    QC: q-chunk size processed per phase (divides SEQ, multiple of 512).
    first_qc: smaller chunk size used at the start of batch 0 to shorten the
    pipeline-fill latency before the first A DMA can start.
    """
    assert D == 128
    assert SEQ % QC == 0 and QC % 512 == 0
    KT = SEQ // 128  # k tiles per batch
    scale = 1.0 / math.sqrt(D)

    def chunk_list(b):
        chunks = [QC] * (SEQ // QC)
        if b == 0 and first_qc and first_qc < QC:
            # ramp up: small chunks first to shorten the pipeline fill
            rest = QC - 2 * first_qc
            ramp = [first_qc, first_qc] + ([rest] if rest else [])
            chunks = ramp + chunks[1:]
        if last_split and b == BPC - 1 and QC >= 1024:
            # ramp down: end on a small chunk to shorten the DMA tail
            chunks = chunks[:-1] + [QC // 2, QC // 2]
        return chunks

    qT_d = nc.declare_dram_parameter("qT", [BPC, D, SEQ], BF16, isOutput=False).ap()
    kT_d = nc.declare_dram_parameter("kT", [BPC, D, SEQ], BF16, isOutput=False).ap()
    v_d = nc.declare_dram_parameter("v", [BPC, SEQ, D], BF16, isOutput=False).ap()
    vo_d = nc.declare_dram_parameter("v_out", [BPC, SEQ, D], F32, isOutput=True).ap()
    # A is computed in bf16 on-device; ship it as bf16 (the host upcast to
    # f32 is lossless), halving the dominant DMA-out traffic.
    aT_d = nc.declare_dram_parameter(
        "aT_out", [BPC, SEQ, SEQ], BF16, isOutput=True
    ).ap()

    with tile.TileContext(nc) as tc, ExitStack() as ctx:
        singles = ctx.enter_context(tc.tile_pool(name="singles", bufs=1))
        qk_pool = ctx.enter_context(tc.tile_pool(name="qk", bufs=2))
        v_pool = ctx.enter_context(tc.tile_pool(name="vp", bufs=2))
        e_pool = ctx.enter_context(tc.tile_pool(name="et", bufs=3 * KT))
        a_pool = ctx.enter_context(tc.tile_pool(name="ast", bufs=5))
        o_pool = ctx.enter_context(tc.tile_pool(name="ost", bufs=3))
        r_pool = ctx.enter_context(tc.tile_pool(name="recip", bufs=3))
        ps_s = ctx.enter_context(
            tc.tile_pool(name="ps_s", bufs=ps_s_bufs, space="PSUM")
        )
        ps_o = ctx.enter_context(
            tc.tile_pool(name="ps_o", bufs=ps_o_bufs, space="PSUM")
        )

        ident = singles.tile([128, 128], F32)
        make_identity(nc, ident)

        aT_v = [aT_d[b].rearrange("(t p) q -> p t q", p=128) for b in range(BPC)]
        vo_v = [vo_d[b].rearrange("(t p) d -> p t d", p=128) for b in range(BPC)]

        class Chunk:
            """Per-chunk state carried across the one-chunk software pipeline."""

            def __init__(self, b, q0, qc, e_tiles, v_sb):
                self.b, self.q0, self.qc = b, q0, qc
                self.e_tiles, self.v_sb = e_tiles, v_sb
                self.JT = qc // 128
                self.rcols = r_pool.tile([128, QC // 128], F32, tag="rcols")
                self.o_st = o_pool.tile([128, QC // 128, D], F32, tag="ost")
                self.rT = r_pool.tile([1, QC], BF16, tag="rT")

        def emit_av_group(st, j):
            """AV matmuls + normalization lead-in for one q-tile of chunk st."""
            po = ps_o.tile([128, D + 1], F32, tag="po")
            for i in range(KT):
                nc.tensor.matmul(
                    po,
                    lhsT=st.e_tiles[i][:, j * 128 : (j + 1) * 128],
                    rhs=st.v_sb[:, i, :],
                    start=(i == 0),
                    stop=(i == KT - 1),
                )
            nc.vector.reciprocal(st.rcols[:, j : j + 1], po[:, D : D + 1])
            nc.vector.tensor_scalar_mul(
                st.o_st[:, j, :], po[:, 0:D], st.rcols[:, j : j + 1]
            )

        def emit_chunk_tail(st):
            """v_out DMA, 1/rowsum broadcast, A^T normalize + stream out."""
            b, q0, qc, JT = st.b, st.q0, st.qc, st.JT
            for j in range(JT):
                pt = ps_o.tile([1, 128], F32, tag="po")
                nc.tensor.transpose(pt, st.rcols[:, j : j + 1], ident)
                nc.vector.tensor_copy(
                    out=st.rT[:, j * 128 : (j + 1) * 128], in_=pt
                )
            nc.sync.dma_start(
                out=vo_v[b][:, q0 // 128 : q0 // 128 + JT, :],
                in_=st.o_st[:, :JT, :],
            )
            rrep = r_pool.tile([128, QC], BF16, tag="rrep")
            nc.gpsimd.partition_broadcast(
                out_ap=rrep[:, :qc], in_ap=st.rT[:, :qc]
            )
            for i in range(0, KT, 2):
                ast = a_pool.tile([128, 2, QC], BF16, tag="ast")
                nc.vector.tensor_mul(
                    ast[:, 0, :qc], st.e_tiles[i][:, :qc], rrep[:, :qc]
                )
                nc.vector.tensor_mul(
                    ast[:, 1, :qc], st.e_tiles[i + 1][:, :qc], rrep[:, :qc]
                )
                nc.sync.dma_start(
                    out=aT_v[b][:, i : i + 2, q0 : q0 + qc],
                    in_=ast[:, :, :qc],
                )

        loaded = {}

        def emit_loads(b):
            qT_sb = qk_pool.tile([128, SEQ], BF16, tag="qT")
            kT_sb = qk_pool.tile([128, SEQ], BF16, tag="kT")
            v_sb = v_pool.tile([128, KT, D + 1], BF16, tag="v")
            nc.sync.dma_start(out=qT_sb, in_=qT_d[b])
            nc.sync.dma_start(out=kT_sb, in_=kT_d[b])
            nc.vector.memset(v_sb[:, :, D : D + 1], 1.0)
            nc.sync.dma_start(
                out=v_sb[:, :, 0:D],
                in_=v_d[b].rearrange("(t p) d -> p t d", p=128),
            )
            loaded[b] = (qT_sb, kT_sb, v_sb)

        def emit_qk_exp(b, q0, qc):
            """QK matmuls + exp for one chunk; returns its pipeline state."""
            if b not in loaded:
                emit_loads(b)
            qT_sb, kT_sb, v_sb = loaded[b]
            e_tiles = []
            mmn = min(qc, 512)
            for i in range(KT):
                ps = ps_s.tile([128, QC], F32, tag="s")
                for c in range(qc // mmn):
                    nc.tensor.matmul(
                        ps[:, c * mmn : (c + 1) * mmn],
                        lhsT=kT_sb[:, i * 128 : (i + 1) * 128],
                        rhs=qT_sb[:, q0 + c * mmn : q0 + (c + 1) * mmn],
                        start=True,
                        stop=True,
                    )
                et = e_pool.tile([128, QC], BF16, tag="et")
                nc.scalar.activation(
                    out=et[:, :qc],
                    in_=ps[:, :qc],
                    func=mybir.ActivationFunctionType.Exp,
                    scale=scale,
                )
                e_tiles.append(et)
            return Chunk(b, q0, qc, e_tiles, v_sb)

        all_chunks = []
        for b in range(BPC):
            q0 = 0
            for qc in chunk_list(b):
                all_chunks.append((b, q0, qc))
                q0 += qc

        # one-chunk software pipeline: QK/exp of chunk m+1 is queued on the
        # engines before the AV/normalize work of chunk m, so the activation
        # engine always has a filled PSUM pipeline to chew on
        st = emit_qk_exp(*all_chunks[0]) if interleave_av else None
        for m in range(len(all_chunks)):
            if interleave_av:
                nxt = (
                    emit_qk_exp(*all_chunks[m + 1])
                    if m + 1 < len(all_chunks)
                    else None
                )
            else:
                st = emit_qk_exp(*all_chunks[m])
            for j in range(st.JT):
                emit_av_group(st, j)
            emit_chunk_tail(st)
            if interleave_av:
                st = nxt
    return nc


_COMPILED = None


def _get_compiled():
    global _COMPILED
    if _COMPILED is None:
        nc = bacc.Bacc("TRN2", target_bir_lowering=False, debug=False)
        import json

        cfg = json.loads(os.environ.get("KERNEL_CFG", "{}"))
        build_attention(
            nc,
            BPC=BPC_FULL,
            SEQ=SEQ_FULL,
            D=D_FULL,
            QC=cfg.get("qc", 1024),
            first_qc=cfg.get("first_qc", 512),
            interleave_av=cfg.get("interleave_av", False),
            last_split=cfg.get("last_split", True),
            ps_s_bufs=cfg.get("ps_s_bufs", 2),
            ps_o_bufs=cfg.get("ps_o_bufs", 4),
        )
        nc.compile()
        _COMPILED = nc
    return _COMPILED


def kernel(query, key, value):
    global LAST_RESULTS
    assert query.shape == (B_FULL, SEQ_FULL, D_FULL), query.shape
    assert key.shape == (B_FULL, SEQ_FULL, D_FULL), key.shape
    assert value.shape == (B_FULL, SEQ_FULL, D_FULL), value.shape

    nc = _get_compiled()

    bf16 = ml_dtypes.bfloat16
    qT = query.transpose(0, 2, 1).astype(bf16)  # [B, D, SEQ]
    kT = key.transpose(0, 2, 1).astype(bf16)  # [B, D, SEQ]
    vb = np.ascontiguousarray(value).astype(bf16)  # [B, SEQ, D]

    in_maps = []
    for c in range(N_CORES):
        sl = slice(c * BPC_FULL, (c + 1) * BPC_FULL)
        in_maps.append(
            {
                "qT": np.ascontiguousarray(qT[sl]),
                "kT": np.ascontiguousarray(kT[sl]),
                "v": vb[sl],
            }
        )

    res = run_bass_kernel_spmd(
        nc,
        in_maps,
        list(range(N_CORES)),
        trace=bool(os.environ.get("KERNEL_TRACE")),
    )
    LAST_RESULTS = res

    v_out = np.concatenate(
        [res.results[i]["v_out"] for i in range(N_CORES)], axis=0
    )
    aT = np.concatenate([res.results[i]["aT_out"] for i in range(N_CORES)], axis=0)
    # lossless bf16 -> f32 upcast on the contiguous array, then a zero-copy
    # transposed view -> [B, Q, K]
    a = aT.astype(np.float32).transpose(0, 2, 1)
    return v_out, a


# revision 43
# speedup vs baseline: 1.0658x; 1.0658x over previous
"""Trainium2 Bass kernel for batched dot-product attention that also returns
the attention-probability matrix.

reference:
    z = einsum("bqd,bkd->bqk", q, k) / sqrt(d)
    a = softmax(z, axis=-1)            # [B, Q, K]  (also an output!)
    v = einsum("bqk,bkd->bqd", a, v)   # [B, Q, D]
    returns (v, a)

Shapes: B=32, Q=K=2048, D=128, fp32 in/out.  8 NeuronCores, batch-sharded
4 batches/core (fully data parallel, no collectives).

Device-side layout trick: everything is computed in the "S^T" layout
(S^T[k, q] tiles with k on partitions) so that
  - the first matmul is  S^T = (K^T)^T-stationary @ Q^T-moving,
  - exp(S^T) tiles feed the A@V matmul directly as stationary operands
    (contraction over k = partitions) with V as the moving operand,
  - an extra all-ones column appended to V yields the softmax denominators
    in the same accumulation (flash-attention style),
  - A^T is written to DRAM [b, k, q]-major and transposed to [b, q, k] on the
    host as a zero-copy numpy view during the gather step.
Q/K arrive pre-transposed (and bf16-cast) from the host staging step, V
arrives bf16; all matmuls run in bf16 (f32 PSUM accumulation).
"""

import math
import os
import sys

sys.path.insert(0, "/opt/trn_rl_repo")

import numpy as np
import ml_dtypes

from contextlib import ExitStack

import concourse.bass as bass  # noqa: F401  (engine types come via nc)
import concourse.tile as tile
from concourse import bacc, mybir
from concourse.bass_utils import run_bass_kernel_spmd
from concourse.masks import make_identity

BF16 = mybir.dt.bfloat16
F32 = mybir.dt.float32

N_CORES = 8
B_FULL, SEQ_FULL, D_FULL = 32, 2048, 128
BPC_FULL = B_FULL // N_CORES  # batches per core

LAST_RESULTS = None  # BassKernelResults of the most recent kernel() call


def build_attention(
    nc,
    BPC,
    SEQ,
    D,
    QC,
    first_qc=None,
    interleave_av=False,
    last_split=True,
    ps_s_bufs=2,
    ps_o_bufs=4,
):
    """Emit the per-core attention kernel into `nc`.

    BPC: batches per core, SEQ: q==k length, D: head dim (must be 128),
    QC: q-chunk size processed per phase (divides SEQ, multiple of 512).
    first_qc: smaller chunk size used at the start of batch 0 to shorten the
    pipeline-fill latency before the first A DMA can start.
    """
    assert D == 128
    assert SEQ % QC == 0 and QC % 512 == 0
    KT = SEQ // 128  # k tiles per batch
    scale = 1.0 / math.sqrt(D)

    def chunk_list(b):
        chunks = [QC] * (SEQ // QC)
        if b == 0 and first_qc and first_qc < QC:
            # ramp up: small chunks first to shorten the pipeline fill
            rest = QC - 2 * first_qc
            ramp = [first_qc, first_qc] + ([rest] if rest else [])
            chunks = ramp + chunks[1:]
        if last_split and b == BPC - 1 and QC >= 1024:
            # ramp down: end on a small chunk to shorten the DMA tail
            chunks = chunks[:-1] + [QC // 2, QC // 2]
        return chunks

    qT_d = nc.declare_dram_parameter("qT", [BPC, D, SEQ], BF16, isOutput=False).ap()
    kT_d = nc.declare_dram_parameter("kT", [BPC, D, SEQ], BF16, isOutput=False).ap()
    v_d = nc.declare_dram_parameter("v", [BPC, SEQ, D], BF16, isOutput=False).ap()
    vo_d = nc.declare_dram_parameter("v_out", [BPC, SEQ, D], F32, isOutput=True).ap()
    # A is computed in bf16 on-device; ship it as bf16 (the host upcast to
    # f32 is lossless), halving the dominant DMA-out traffic.
    aT_d = nc.declare_dram_parameter(
        "aT_out", [BPC, SEQ, SEQ], BF16, isOutput=True
    ).ap()

    with tile.TileContext(nc) as tc, ExitStack() as ctx:
        singles = ctx.enter_context(tc.tile_pool(name="singles", bufs=1))
        qk_pool = ctx.enter_context(tc.tile_pool(name="qk", bufs=2))
        v_pool = ctx.enter_context(tc.tile_pool(name="vp", bufs=2))
        e_pool = ctx.enter_context(tc.tile_pool(name="et", bufs=3 * KT))
        a_pool = ctx.enter_context(tc.tile_pool(name="ast", bufs=5))
        o_pool = ctx.enter_context(tc.tile_pool(name="ost", bufs=3))
        r_pool = ctx.enter_context(tc.tile_pool(name="recip", bufs=3))
        ps_s = ctx.enter_context(
            tc.tile_pool(name="ps_s", bufs=ps_s_bufs, space="PSUM")
        )
        ps_o = ctx.enter_context(
            tc.tile_pool(name="ps_o", bufs=ps_o_bufs, space="PSUM")
        )

        ident = singles.tile([128, 128], F32)
        make_identity(nc, ident)

        aT_v = [aT_d[b].rearrange("(t p) q -> p t q", p=128) for b in range(BPC)]
        vo_v = [vo_d[b].rearrange("(t p) d -> p t d", p=128) for b in range(BPC)]

        class Chunk:
            """Per-chunk state carried across the one-chunk software pipeline."""

            def __init__(self, b, q0, qc, e_tiles, v_sb):
                self.b, self.q0, self.qc = b, q0, qc
                self.e_tiles, self.v_sb = e_tiles, v_sb
                self.JT = qc // 128
                self.rcols = r_pool.tile([128, QC // 128], F32, tag="rcols")
                self.o_st = o_pool.tile([128, QC // 128, D], F32, tag="ost")
                self.rT = r_pool.tile([1, QC], BF16, tag="rT")

        def emit_av_phase(st):
            """AV matmuls for all q-tiles of chunk st, i-major so every
            accumulator advances in lockstep with the exp stream (two q-tile
            accumulators share one PSUM bank: 2*129 f32 = 1032B <= 2KB)."""
            npair = (st.JT + 1) // 2
            po_tiles = []
            for _p in range(npair):
                po_pair = ps_o.tile([128, 2, D + 1], F32, tag="po")
                po_tiles.append(po_pair)
            for i in range(KT):
                for j in range(st.JT):
                    # start=True clears has_written for the WHOLE bank, so only
                    # the first matmul per bank may set it; the sibling region's
                    # i==0 matmul then lands as an overwrite (bits cleared)
                    nc.tensor.matmul(
                        po_tiles[j // 2][:, j % 2, :],
                        lhsT=st.e_tiles[i][:, j * 128 : (j + 1) * 128],
                        rhs=st.v_sb[:, i, :],
                        start=(i == 0 and j % 2 == 0),
                        stop=(i == KT - 1 and j % 2 == 1),
                        skip_group_check=True,
                    )
            for j in range(st.JT):
                po = po_tiles[j // 2]
                nc.vector.reciprocal(
                    st.rcols[:, j : j + 1], po[:, j % 2, D : D + 1]
                )
                nc.vector.tensor_scalar_mul(
                    st.o_st[:, j, :], po[:, j % 2, 0:D], st.rcols[:, j : j + 1]
                )

        def emit_chunk_tail(st):
            """v_out DMA, 1/rowsum broadcast, A^T normalize + stream out."""
            b, q0, qc, JT = st.b, st.q0, st.qc, st.JT
            for j in range(JT):
                pt = ps_o.tile([1, 128], F32, tag="po")
                nc.tensor.transpose(pt, st.rcols[:, j : j + 1], ident)
                nc.vector.tensor_copy(
                    out=st.rT[:, j * 128 : (j + 1) * 128], in_=pt
                )
            nc.sync.dma_start(
                out=vo_v[b][:, q0 // 128 : q0 // 128 + JT, :],
                in_=st.o_st[:, :JT, :],
            )
            rrep = r_pool.tile([128, QC], BF16, tag="rrep")
            nc.gpsimd.partition_broadcast(
                out_ap=rrep[:, :qc], in_ap=st.rT[:, :qc]
            )
            for i in range(0, KT, 2):
                ast = a_pool.tile([128, 2, QC], BF16, tag="ast")
                nc.vector.tensor_mul(
                    ast[:, 0, :qc], st.e_tiles[i][:, :qc], rrep[:, :qc]
                )
                nc.vector.tensor_mul(
                    ast[:, 1, :qc], st.e_tiles[i + 1][:, :qc], rrep[:, :qc]
                )
                nc.sync.dma_start(
                    out=aT_v[b][:, i : i + 2, q0 : q0 + qc],
                    in_=ast[:, :, :qc],
                )

        loaded = {}

        def emit_loads(b):
            qT_sb = qk_pool.tile([128, SEQ], BF16, tag="qT")
            kT_sb = qk_pool.tile([128, SEQ], BF16, tag="kT")
            v_sb = v_pool.tile([128, KT, D + 1], BF16, tag="v")
            nc.sync.dma_start(out=qT_sb, in_=qT_d[b])
            nc.sync.dma_start(out=kT_sb, in_=kT_d[b])
            nc.vector.memset(v_sb[:, :, D : D + 1], 1.0)
            nc.sync.dma_start(
                out=v_sb[:, :, 0:D],
                in_=v_d[b].rearrange("(t p) d -> p t d", p=128),
            )
            loaded[b] = (qT_sb, kT_sb, v_sb)

        def emit_qk_exp(b, q0, qc):
            """QK matmuls + exp for one chunk; returns its pipeline state."""
            if b not in loaded:
                emit_loads(b)
            qT_sb, kT_sb, v_sb = loaded[b]
            e_tiles = []
            mmn = min(qc, 512)
            for i in range(KT):
                ps = ps_s.tile([128, QC], F32, tag="s")
                for c in range(qc // mmn):
                    nc.tensor.matmul(
                        ps[:, c * mmn : (c + 1) * mmn],
                        lhsT=kT_sb[:, i * 128 : (i + 1) * 128],
                        rhs=qT_sb[:, q0 + c * mmn : q0 + (c + 1) * mmn],
                        start=True,
                        stop=True,
                    )
                et = e_pool.tile([128, QC], BF16, tag="et")
                nc.scalar.activation(
                    out=et[:, :qc],
                    in_=ps[:, :qc],
                    func=mybir.ActivationFunctionType.Exp,
                    scale=scale,
                )
                e_tiles.append(et)
            return Chunk(b, q0, qc, e_tiles, v_sb)

        all_chunks = []
        for b in range(BPC):
            q0 = 0
            for qc in chunk_list(b):
                all_chunks.append((b, q0, qc))
                q0 += qc

        # one-chunk software pipeline: QK/exp of chunk m+1 is queued on the
        # engines before the AV/normalize work of chunk m, so the activation
        # engine always has a filled PSUM pipeline to chew on
        st = emit_qk_exp(*all_chunks[0]) if interleave_av else None
        for m in range(len(all_chunks)):
            if interleave_av:
                nxt = (
                    emit_qk_exp(*all_chunks[m + 1])
                    if m + 1 < len(all_chunks)
                    else None
                )
            else:
                st = emit_qk_exp(*all_chunks[m])
            emit_av_phase(st)
            emit_chunk_tail(st)
            if interleave_av:
                st = nxt
    return nc


_COMPILED = None


def _get_compiled():
    global _COMPILED
    if _COMPILED is None:
        nc = bacc.Bacc("TRN2", target_bir_lowering=False, debug=False)
        import json

        cfg = json.loads(os.environ.get("KERNEL_CFG", "{}"))
        build_attention(
            nc,
            BPC=BPC_FULL,
            SEQ=SEQ_FULL,
            D=D_FULL,
            QC=cfg.get("qc", 1024),
            first_qc=cfg.get("first_qc", 512),
            interleave_av=cfg.get("interleave_av", False),
            last_split=cfg.get("last_split", True),
            ps_s_bufs=cfg.get("ps_s_bufs", 2),
            ps_o_bufs=cfg.get("ps_o_bufs", 4),
        )
        nc.compile()
        _COMPILED = nc
    return _COMPILED


def kernel(query, key, value):
    global LAST_RESULTS
    assert query.shape == (B_FULL, SEQ_FULL, D_FULL), query.shape
    assert key.shape == (B_FULL, SEQ_FULL, D_FULL), key.shape
    assert value.shape == (B_FULL, SEQ_FULL, D_FULL), value.shape

    nc = _get_compiled()

    bf16 = ml_dtypes.bfloat16
    qT = query.transpose(0, 2, 1).astype(bf16)  # [B, D, SEQ]
    kT = key.transpose(0, 2, 1).astype(bf16)  # [B, D, SEQ]
    vb = np.ascontiguousarray(value).astype(bf16)  # [B, SEQ, D]

    in_maps = []
    for c in range(N_CORES):
        sl = slice(c * BPC_FULL, (c + 1) * BPC_FULL)
        in_maps.append(
            {
                "qT": np.ascontiguousarray(qT[sl]),
                "kT": np.ascontiguousarray(kT[sl]),
                "v": vb[sl],
            }
        )

    res = run_bass_kernel_spmd(
        nc,
        in_maps,
        list(range(N_CORES)),
        trace=bool(os.environ.get("KERNEL_TRACE")),
    )
    LAST_RESULTS = res

    v_out = np.concatenate(
        [res.results[i]["v_out"] for i in range(N_CORES)], axis=0
    )
    aT = np.concatenate([res.results[i]["aT_out"] for i in range(N_CORES)], axis=0)
    # lossless bf16 -> f32 upcast on the contiguous array, then a zero-copy
    # transposed view -> [B, Q, K]
    a = aT.astype(np.float32).transpose(0, 2, 1)
    return v_out, a


# revision 44
# speedup vs baseline: 1.0852x; 1.0182x over previous
"""Trainium2 Bass kernel for batched dot-product attention that also returns
the attention-probability matrix.

reference:
    z = einsum("bqd,bkd->bqk", q, k) / sqrt(d)
    a = softmax(z, axis=-1)            # [B, Q, K]  (also an output!)
    v = einsum("bqk,bkd->bqd", a, v)   # [B, Q, D]
    returns (v, a)

Shapes: B=32, Q=K=2048, D=128, fp32 in/out.  8 NeuronCores, batch-sharded
4 batches/core (fully data parallel, no collectives).

Device-side layout trick: everything is computed in the "S^T" layout
(S^T[k, q] tiles with k on partitions) so that
  - the first matmul is  S^T = (K^T)^T-stationary @ Q^T-moving,
  - exp(S^T) tiles feed the A@V matmul directly as stationary operands
    (contraction over k = partitions) with V as the moving operand,
  - an extra all-ones column appended to V yields the softmax denominators
    in the same accumulation (flash-attention style),
  - A^T is written to DRAM [b, k, q]-major and transposed to [b, q, k] on the
    host as a zero-copy numpy view during the gather step.
Q/K arrive pre-transposed (and bf16-cast) from the host staging step, V
arrives bf16; all matmuls run in bf16 (f32 PSUM accumulation).
"""

import math
import os
import sys

sys.path.insert(0, "/opt/trn_rl_repo")

import numpy as np
import ml_dtypes

from contextlib import ExitStack

import concourse.bass as bass  # noqa: F401  (engine types come via nc)
import concourse.tile as tile
from concourse import bacc, mybir
from concourse.bass_utils import run_bass_kernel_spmd
from concourse.masks import make_identity

BF16 = mybir.dt.bfloat16
F32 = mybir.dt.float32

N_CORES = 8
B_FULL, SEQ_FULL, D_FULL = 32, 2048, 128
BPC_FULL = B_FULL // N_CORES  # batches per core

LAST_RESULTS = None  # BassKernelResults of the most recent kernel() call


def build_attention(
    nc,
    BPC,
    SEQ,
    D,
    QC,
    first_qc=None,
    interleave_av=False,
    last_split=True,
    ps_s_bufs=2,
    ps_o_bufs=4,
):
    """Emit the per-core attention kernel into `nc`.

    BPC: batches per core, SEQ: q==k length, D: head dim (must be 128),
    QC: q-chunk size processed per phase (divides SEQ, multiple of 512).
    first_qc: smaller chunk size used at the start of batch 0 to shorten the
    pipeline-fill latency before the first A DMA can start.
    """
    assert D == 128
    assert SEQ % QC == 0 and QC % 512 == 0
    KT = SEQ // 128  # k tiles per batch
    scale = 1.0 / math.sqrt(D)

    def chunk_list(b):
        chunks = [QC] * (SEQ // QC)
        if b == 0 and first_qc and first_qc < QC:
            # ramp up: small chunks first to shorten the pipeline fill
            rest = QC - 2 * first_qc
            ramp = [first_qc, first_qc] + ([rest] if rest else [])
            chunks = ramp + chunks[1:]
        if last_split and b == BPC - 1 and QC >= 1024:
            # ramp down: end on a small chunk to shorten the DMA tail
            chunks = chunks[:-1] + [QC // 2, QC // 2]
        return chunks

    qT_d = nc.declare_dram_parameter("qT", [BPC, D, SEQ], BF16, isOutput=False).ap()
    kT_d = nc.declare_dram_parameter("kT", [BPC, D, SEQ], BF16, isOutput=False).ap()
    v_d = nc.declare_dram_parameter("v", [BPC, SEQ, D], BF16, isOutput=False).ap()
    vo_d = nc.declare_dram_parameter("v_out", [BPC, SEQ, D], F32, isOutput=True).ap()
    # A is computed in bf16 on-device; ship it as bf16 (the host upcast to
    # f32 is lossless), halving the dominant DMA-out traffic.
    aT_d = nc.declare_dram_parameter(
        "aT_out", [BPC, SEQ, SEQ], BF16, isOutput=True
    ).ap()

    with tile.TileContext(nc) as tc, ExitStack() as ctx:
        singles = ctx.enter_context(tc.tile_pool(name="singles", bufs=1))
        qk_pool = ctx.enter_context(tc.tile_pool(name="qk", bufs=2))
        v_pool = ctx.enter_context(tc.tile_pool(name="vp", bufs=2))
        e_pool = ctx.enter_context(tc.tile_pool(name="et", bufs=3 * KT + KT // 2))
        a_pool = ctx.enter_context(tc.tile_pool(name="ast", bufs=5))
        o_pool = ctx.enter_context(tc.tile_pool(name="ost", bufs=4))
        r_pool = ctx.enter_context(tc.tile_pool(name="recip", bufs=3))
        ps_s = ctx.enter_context(
            tc.tile_pool(name="ps_s", bufs=ps_s_bufs, space="PSUM")
        )
        ps_o = ctx.enter_context(
            tc.tile_pool(name="ps_o", bufs=ps_o_bufs, space="PSUM")
        )

        ident = singles.tile([128, 128], F32)
        make_identity(nc, ident)

        aT_v = [aT_d[b].rearrange("(t p) q -> p t q", p=128) for b in range(BPC)]
        vo_v = [vo_d[b].rearrange("(t p) d -> p t d", p=128) for b in range(BPC)]

        class Chunk:
            """Per-chunk state carried across the one-chunk software pipeline."""

            def __init__(self, b, q0, qc, e_tiles, v_sb):
                self.b, self.q0, self.qc = b, q0, qc
                self.e_tiles, self.v_sb = e_tiles, v_sb
                self.JT = qc // 128
                self.rcols = r_pool.tile([128, QC // 128], F32, tag="rcols")
                self.o_st = o_pool.tile([128, QC // 128, D], F32, tag="ost")
                self.rT = r_pool.tile([1, QC], BF16, tag="rT")

        def emit_av_phase(st):
            """AV matmuls for all q-tiles of chunk st, i-major so every
            accumulator advances in lockstep with the exp stream (two q-tile
            accumulators share one PSUM bank: 2*129 f32 = 1032B <= 2KB)."""
            npair = (st.JT + 1) // 2
            po_tiles = []
            for _p in range(npair):
                po_pair = ps_o.tile([128, 2, D + 1], F32, tag="po")
                po_tiles.append(po_pair)
            for i in range(KT):
                for j in range(st.JT):
                    # start=True clears has_written for the WHOLE bank, so only
                    # the first matmul per bank may set it; the sibling region's
                    # i==0 matmul then lands as an overwrite (bits cleared)
                    nc.tensor.matmul(
                        po_tiles[j // 2][:, j % 2, :],
                        lhsT=st.e_tiles[i][:, j * 128 : (j + 1) * 128],
                        rhs=st.v_sb[:, i, :],
                        start=(i == 0 and j % 2 == 0),
                        stop=(i == KT - 1 and j % 2 == 1),
                        skip_group_check=True,
                    )
            for j in range(st.JT):
                po = po_tiles[j // 2]
                nc.vector.reciprocal(
                    st.rcols[:, j : j + 1], po[:, j % 2, D : D + 1]
                )
                nc.vector.tensor_scalar_mul(
                    st.o_st[:, j, :], po[:, j % 2, 0:D], st.rcols[:, j : j + 1]
                )

        def emit_chunk_tail(st):
            """v_out DMA, 1/rowsum broadcast, A^T normalize + stream out."""
            b, q0, qc, JT = st.b, st.q0, st.qc, st.JT
            for j in range(JT):
                pt = ps_o.tile([1, 128], F32, tag="po")
                nc.tensor.transpose(pt, st.rcols[:, j : j + 1], ident)
                nc.vector.tensor_copy(
                    out=st.rT[:, j * 128 : (j + 1) * 128], in_=pt
                )
            nc.sync.dma_start(
                out=vo_v[b][:, q0 // 128 : q0 // 128 + JT, :],
                in_=st.o_st[:, :JT, :],
            )
            rrep = r_pool.tile([128, QC], BF16, tag="rrep")
            nc.gpsimd.partition_broadcast(
                out_ap=rrep[:, :qc], in_ap=st.rT[:, :qc]
            )
            for i in range(0, KT, 2):
                ast = a_pool.tile([128, 2, QC], BF16, tag="ast")
                nc.vector.tensor_mul(
                    ast[:, 0, :qc], st.e_tiles[i][:, :qc], rrep[:, :qc]
                )
                nc.vector.tensor_mul(
                    ast[:, 1, :qc], st.e_tiles[i + 1][:, :qc], rrep[:, :qc]
                )
                nc.sync.dma_start(
                    out=aT_v[b][:, i : i + 2, q0 : q0 + qc],
                    in_=ast[:, :, :qc],
                )

        loaded = {}

        def emit_loads(b):
            qT_sb = qk_pool.tile([128, SEQ], BF16, tag="qT")
            kT_sb = qk_pool.tile([128, SEQ], BF16, tag="kT")
            v_sb = v_pool.tile([128, KT, D + 1], BF16, tag="v")
            nc.sync.dma_start(out=qT_sb, in_=qT_d[b])
            nc.sync.dma_start(out=kT_sb, in_=kT_d[b])
            nc.vector.memset(v_sb[:, :, D : D + 1], 1.0)
            nc.sync.dma_start(
                out=v_sb[:, :, 0:D],
                in_=v_d[b].rearrange("(t p) d -> p t d", p=128),
            )
            loaded[b] = (qT_sb, kT_sb, v_sb)

        def emit_qk_exp(b, q0, qc):
            """QK matmuls + exp for one chunk; returns its pipeline state."""
            if b not in loaded:
                emit_loads(b)
            qT_sb, kT_sb, v_sb = loaded[b]
            e_tiles = []
            mmn = min(qc, 512)
            for i in range(KT):
                ps = ps_s.tile([128, QC], F32, tag="s")
                for c in range(qc // mmn):
                    nc.tensor.matmul(
                        ps[:, c * mmn : (c + 1) * mmn],
                        lhsT=kT_sb[:, i * 128 : (i + 1) * 128],
                        rhs=qT_sb[:, q0 + c * mmn : q0 + (c + 1) * mmn],
                        start=True,
                        stop=True,
                    )
                et = e_pool.tile([128, QC], BF16, tag="et")
                nc.scalar.activation(
                    out=et[:, :qc],
                    in_=ps[:, :qc],
                    func=mybir.ActivationFunctionType.Exp,
                    scale=scale,
                )
                e_tiles.append(et)
            return Chunk(b, q0, qc, e_tiles, v_sb)

        all_chunks = []
        for b in range(BPC):
            q0 = 0
            for qc in chunk_list(b):
                all_chunks.append((b, q0, qc))
                q0 += qc

        # one-chunk software pipeline: QK/exp of chunk m+1 is queued on the
        # engines before the AV/normalize work of chunk m, so the activation
        # engine always has a filled PSUM pipeline to chew on
        st = emit_qk_exp(*all_chunks[0]) if interleave_av else None
        for m in range(len(all_chunks)):
            if interleave_av:
                nxt = (
                    emit_qk_exp(*all_chunks[m + 1])
                    if m + 1 < len(all_chunks)
                    else None
                )
            else:
                st = emit_qk_exp(*all_chunks[m])
            emit_av_phase(st)
            emit_chunk_tail(st)
            if interleave_av:
                st = nxt
    return nc


_COMPILED = None


def _get_compiled():
    global _COMPILED
    if _COMPILED is None:
        nc = bacc.Bacc("TRN2", target_bir_lowering=False, debug=False)
        import json

        cfg = json.loads(os.environ.get("KERNEL_CFG", "{}"))
        build_attention(
            nc,
            BPC=BPC_FULL,
            SEQ=SEQ_FULL,
            D=D_FULL,
            QC=cfg.get("qc", 1024),
            first_qc=cfg.get("first_qc", 512),
            interleave_av=cfg.get("interleave_av", False),
            last_split=cfg.get("last_split", True),
            ps_s_bufs=cfg.get("ps_s_bufs", 2),
            ps_o_bufs=cfg.get("ps_o_bufs", 4),
        )
        nc.compile()
        _COMPILED = nc
    return _COMPILED


def kernel(query, key, value):
    global LAST_RESULTS
    assert query.shape == (B_FULL, SEQ_FULL, D_FULL), query.shape
    assert key.shape == (B_FULL, SEQ_FULL, D_FULL), key.shape
    assert value.shape == (B_FULL, SEQ_FULL, D_FULL), value.shape

    nc = _get_compiled()

    bf16 = ml_dtypes.bfloat16
    qT = query.transpose(0, 2, 1).astype(bf16)  # [B, D, SEQ]
    kT = key.transpose(0, 2, 1).astype(bf16)  # [B, D, SEQ]
    vb = np.ascontiguousarray(value).astype(bf16)  # [B, SEQ, D]

    in_maps = []
    for c in range(N_CORES):
        sl = slice(c * BPC_FULL, (c + 1) * BPC_FULL)
        in_maps.append(
            {
                "qT": np.ascontiguousarray(qT[sl]),
                "kT": np.ascontiguousarray(kT[sl]),
                "v": vb[sl],
            }
        )

    res = run_bass_kernel_spmd(
        nc,
        in_maps,
        list(range(N_CORES)),
        trace=bool(os.environ.get("KERNEL_TRACE")),
    )
    LAST_RESULTS = res

    v_out = np.concatenate(
        [res.results[i]["v_out"] for i in range(N_CORES)], axis=0
    )
    aT = np.concatenate([res.results[i]["aT_out"] for i in range(N_CORES)], axis=0)
    # lossless bf16 -> f32 upcast on the contiguous array, then a zero-copy
    # transposed view -> [B, Q, K]
    a = aT.astype(np.float32).transpose(0, 2, 1)
    return v_out, a
